# revision 15
# baseline (speedup 1.0000x reference)
"""Trainium2 Bass kernel for nn_Attend_62534723830373.

Reference computation (note: q is UNUSED by the reference):
    scores = einsum('bhid,bhjd->bhij', k, v) * (1/sqrt(128))
    scores = causal_mask(scores)            # strictly-upper masked
    attn   = softmax(scores, axis=-1)
    out    = einsum('bhij,bhjd->bhid', attn, v)

Shapes: [b=2, h=16, s=2048, d=128] fp32. b*h = 32 head-slices sharded
4-per-core across 8 NeuronCores (data/head parallel, no collectives).

Per-head dataflow on one core (matmul chain in bf16, fp32 accumulate,
measured ~3e-3 scale-relative absmax vs the fp32 reference):
  - SWDGE cast-load K, V (fp32 HBM -> bf16 SBUF, natural layout), the
    first 4 row-blocks in their own chunk so compute starts early.
  - Just-in-time per 512-wide i-chunk: transpose the chunk's four
    128x128 blocks of K and V on the PE (bf16 transpose + DVE copy) to
    build KT[d, s] / VT[d, s], and assemble [V | ones] (129 cols).
  - For each i-chunk, j-block pairs share one 1024-wide (2-bank) PSUM
    score tile and ONE exp instruction (halves ACT's ~293ns fixed cost
    per instruction), emitted with one-pair lookahead so the PE always
    has score matmuls in flight:
      S^T[j, i] = (VT_blk).T @ KT_slice        (PE, contraction d)
      E = exp(SCALE * S^T)                     (ACT, PSUM -> SBUF bf16)
      diag block: E *= upper-tri 0/1 mask      (DVE)
      psum_o[i-blk] += E_slice.T @ [V_blk | 1] (PE, contraction j)
    The ones column makes column 128 of each accumulator the softmax
    denominator - numerator and denominator in one accumulation, and
    the denominator sees the same bf16 weights so quantization largely
    cancels in the normalization. Two accumulators share each PSUM
    bank; since matmul start=True clears has_written bank-wide, each
    bank is a single accumulation group (start only on the bank's first
    write, stop on its last; per-element has_written makes the second
    accumulator's first write an overwrite).
  - out = psum_o[:, 0:128] * (1 / psum_o[:, 128])  (DVE recip + mul),
    stored per i-chunk so the final DMA is small.

kernel(**inputs) takes FULL unsharded inputs and returns the FULL output.
"""

import numpy as np

B, H, S, D = 2, 16, 2048, 128
N_CORES = 8
HPC = (B * H) // N_CORES  # heads per core = 4
NB = S // 128             # 16 j/i blocks per head
NCH = S // 512            # 4 i-chunks per head
SCALE = 0.08838834764831845
C0 = 1.5                  # constant subtracted inside every exp (softmax-invariant)

FP8_MM2 = True

_CACHED_NC = None


def _build_nc():
    import concourse.bass as bass
    import concourse.mybir as mybir
    import concourse.tile as tile
    from concourse import bacc
    from concourse.masks import make_identity, make_upper_triangular
    from contextlib import ExitStack

    f32 = mybir.dt.float32
    bf16 = mybir.dt.bfloat16
    u8 = mybir.dt.uint8
    f8e4 = mybir.dt.float8e4
    Exp = mybir.ActivationFunctionType.Exp
    DoubleRow = mybir.MatmulPerfMode.DoubleRow

    nc = bacc.Bacc("TRN2", num_devices=N_CORES, debug=False)
    kd = nc.dram_tensor("k", [HPC, S, D], f32, kind="ExternalInput")
    vd = nc.dram_tensor("v", [HPC, S, D], f32, kind="ExternalInput")
    od = nc.dram_tensor("out", [HPC, S, D], f32, kind="ExternalOutput")

    with tile.TileContext(nc) as tc, ExitStack() as ctx:
        const = ctx.enter_context(tc.tile_pool(name="const", bufs=1))
        loadp = ctx.enter_context(tc.tile_pool(name="load", bufs=2))
        ktp = ctx.enter_context(tc.tile_pool(name="kt", bufs=2))
        expp = ctx.enter_context(tc.tile_pool(name="expp", bufs=4))
        exp8p = ctx.enter_context(tc.tile_pool(name="exp8p", bufs=4))
        outp = ctx.enter_context(tc.tile_pool(name="outp", bufs=2))
        smallp = ctx.enter_context(tc.tile_pool(name="small", bufs=4))
        # 2 x 3-bank score tiles + 2 x 1-bank accumulator tiles = 8 banks.
        # The per-chunk transpose staging tile comes from the ps pool too
        # (same tag, viewed as bf16), so no separate pt pool is needed.
        ps_pool = ctx.enter_context(tc.tile_pool(name="ps", bufs=2, space="PSUM"))
        po_pool = ctx.enter_context(tc.tile_pool(name="po", bufs=2, space="PSUM"))

        trimask_f32 = const.tile([128, 128], f32, tag="trimask_f32")
        make_upper_triangular(nc, trimask_f32[:, :], val=1.0, diag=True)
        trimask = const.tile([128, 128], bf16, tag="trimask")
        nc.vector.tensor_copy(trimask[:, :], trimask_f32[:, :])
        onesf32 = const.tile([128, NB], f32, tag="onesf32")
        nc.gpsimd.memset(onesf32[:, :], 1.0)
        identbf = const.tile([128, 128], bf16, tag="identbf")
        make_identity(nc, identbf[:, :])
        biasc0 = const.tile([128, 1], f32, tag="biasc0")
        nc.vector.memset(biasc0[:, :], -C0)
        # warmup exp so ACT's one-time table load happens during startup
        # instead of on the first real score tile's critical path
        warm = const.tile([128, 1], f32, tag="warm")
        nc.scalar.activation(warm[:, :], onesf32[:, 0:1], Exp, scale=SCALE)

        for h in range(HPC):
            # ---- loads: fp32 HBM -> bf16 SBUF (SWDGE cast), natural ----
            knat = loadp.tile([128, NB, 128], bf16, tag="knat")
            vnat = loadp.tile([128, NB, 128], bf16, tag="vnat")
            vones = loadp.tile([128, NB, 129], bf16, tag="vones")
            vones8 = loadp.tile([128, 12, 129], f8e4, tag="vones8")
            KT3 = ktp.tile([128, NB, 128], bf16, tag="KT")
            VT3 = ktp.tile([128, NB, 128], bf16, tag="VT")
            kview = kd.ap()[h].rearrange("(n p) d -> p n d", p=128)
            vview = vd.ap()[h].rearrange("(n p) d -> p n d", p=128)
            # first 4 blocks in their own chunk so chunk-0 compute can
            # start early (each SWDGE dma_start costs ~1.4us of Q7 issue,
            # so only two chunks per matrix)
            for c0, c1 in ((0, 4), (4, 16)):
                sl = slice(c0, c1)
                nc.gpsimd.dma_start(knat[:, sl, :], kview[:, sl, :])
                nc.gpsimd.dma_start(vnat[:, sl, :], vview[:, sl, :])
            if FP8_MM2:
                nc.gpsimd.memset(vones8[:, :, 128], 1.0)
                nc.gpsimd.dma_start(vones8[:, 0:12, 0:128], vview[:, 0:12, :])
            KT = KT3.rearrange("p n d -> p (n d)")
            VT = VT3.rearrange("p n d -> p (n d)")

            out_sb = outp.tile([128, NB, 128], f32, tag="out_sb")

            # ---- main causal attention loop ----
            for ci in range(NCH):
                i0b = 4 * ci              # first i-block of chunk
                iend = (i0b + 4) * 128
                # just-in-time per chunk: [V | ones] columns and PE
                # transposes (+ DVE copies) for blocks 4ci..4ci+3; spreads
                # the transpose bursts across the head instead of one
                # stall-prone prologue burst
                slc = slice(i0b, i0b + 4)
                nc.vector.tensor_copy(vones[:, slc, 0:128], vnat[:, slc, :])
                nc.vector.tensor_copy(vones[:, slc, 128], onesf32[:, slc])
                pt = ps_pool.tile([128, 1536], f32, tag="ps", name=f"pt_{h}_{ci}")
                ptb = pt[:, :].bitcast(bf16)  # [128, 3072] bf16 view
                for u in range(4):
                    bn = i0b + u
                    nc.tensor.transpose(
                        ptb[:, 128 * u : 128 * u + 128], knat[:, bn, :], identbf[:, :]
                    )
                    nc.tensor.transpose(
                        ptb[:, 512 + 128 * u : 640 + 128 * u],
                        vnat[:, bn, :],
                        identbf[:, :],
                    )
                nc.vector.tensor_copy(KT3[:, slc, :], ptb[:, 0:512])
                nc.vector.tensor_copy(VT3[:, slc, :], ptb[:, 512:1024])
                po = [
                    po_pool.tile([128, 258], f32, tag="po", name=f"po_{h}_{ci}_{u}")
                    for u in range(2)
                ]

                def po_ap(bi):
                    u = bi - i0b
                    return po[u // 2][:, (u % 2) * 129 : (u % 2) * 129 + 129]

                # j-blocks are batched into exp GROUPS: off-diagonal
                # ("full") blocks in runs of up to 3 (512 cols each,
                # bank-aligned), and the 4 diagonal blocks packed into
                # exactly 1280 cols (512@0, 384@512, 128@896, 256@1024 -
                # every matmul output stays inside one PSUM bank, which
                # the ISA requires). One exp instruction per group cuts
                # the serial MM1 -> exp -> MM2 round trips per head from
                # 20 to 13 and ACT's fixed cost per instruction.
                # Full groups write fp8e4 weights; their MM2 runs block
                # pairs as DoubleRow (K=256 at the fp8 2x rate).
                # Emitted with one-group lookahead as before.
                groups = []  # (is_full, [(bj, ista, n1, off), ...])
                run = []
                for bj in range(i0b):
                    run.append((bj, i0b * 128, 512, 512 * len(run)))
                    if len(run) == 3:
                        groups.append((FP8_MM2, run))
                        run = []
                if run:
                    groups.append((FP8_MM2, run))
                dmem = []
                for bj, off in (
                    (i0b, 0),
                    (i0b + 1, 512),
                    (i0b + 3, 896),
                    (i0b + 2, 1024),
                ):
                    ist = bj * 128
                    dmem.append((bj, ist, iend - ist, off))
                groups.append((False, dmem))

                pending = None  # awaiting MM2 emission
                for gi in range(len(groups) + 1):
                    cur = None
                    if gi < len(groups):
                        gfull, mem = groups[gi]
                        gwid = mem[-1][3] + mem[-1][2]
                        ps = ps_pool.tile([128, 1536], f32, tag="ps")
                        for bj, ist, n1, off in mem:
                            nc.tensor.matmul(
                                ps[:, off : off + n1],
                                VT[:, bj * 128 : (bj + 1) * 128],
                                KT[:, ist:iend],
                                start=True,
                                stop=True,
                            )
                        if gfull:
                            ex8 = exp8p.tile([128, 1536], u8, tag="ex8")
                            nc.scalar.activation(
                                ex8[:, 0:gwid].bitcast(f8e4),
                                ps[:, 0:gwid],
                                Exp,
                                bias=biasc0[:, :],
                                scale=SCALE,
                            )
                            cur = (True, mem, ex8)
                        else:
                            ex = expp.tile([128, 1536], bf16, tag="ex")
                            nc.scalar.activation(
                                ex[:, 0:gwid],
                                ps[:, 0:gwid],
                                Exp,
                                bias=biasc0[:, :],
                                scale=SCALE,
                            )
                            for bj, ist, n1, off in mem:
                                if bj < i0b:
                                    continue
                                # diagonal: zero the masked (j > i) triangle
                                nc.vector.tensor_mul(
                                    ex[:, off : off + 128],
                                    ex[:, off : off + 128],
                                    trimask[:, :],
                                )
                            cur = (False, mem, ex)
                    if pending is not None:
                        pfull, pmem, pex = pending
                        if pfull:
                            mi = 0
                            while mi < len(pmem):
                                if mi + 1 < len(pmem):
                                    # DoubleRow over blocks bj, bj+1
                                    bj0, ist0, n10, off0 = pmem[mi]
                                    ex3 = pex[:, off0 : off0 + 1024].rearrange(
                                        "p (t w) -> p t w", t=2
                                    )
                                    for bi in range(i0b, i0b + 4):
                                        c0_ = (bi - i0b) * 128
                                        nc.tensor.matmul(
                                            po_ap(bi),
                                            ex3[:, :, c0_ : c0_ + 128].bitcast(f8e4),
                                            vones8[:, bj0 : bj0 + 2, :],
                                            start=(
                                                bj0 == 0 and (bi - i0b) % 2 == 0
                                            ),
                                            stop=False,
                                            perf_mode=DoubleRow,
                                            skip_group_check=True,
                                        )
                                    mi += 2
                                else:
                                    bj0, ist0, n10, off0 = pmem[mi]
                                    for bi in range(i0b, i0b + 4):
                                        c0_ = off0 + (bi - i0b) * 128
                                        nc.tensor.matmul(
                                            po_ap(bi),
                                            pex[:, c0_ : c0_ + 128].bitcast(f8e4),
                                            vones8[:, bj0, :],
                                            start=(
                                                bj0 == 0 and (bi - i0b) % 2 == 0
                                            ),
                                            stop=False,
                                            skip_group_check=True,
                                        )
                                    mi += 1
                        else:
                            for bj, ist, n1, off in pmem:
                                for bi in range(ist // 128, i0b + 4):
                                    c0_ = off + bi * 128 - ist
                                    nc.tensor.matmul(
                                        po_ap(bi),
                                        pex[:, c0_ : c0_ + 128],
                                        vones[:, bj, :],
                                        start=(bj == 0 and (bi - i0b) % 2 == 0),
                                        stop=(bj == bi and (bi - i0b) % 2 == 1),
                                        skip_group_check=True,
                                    )
                    pending = cur
                for u in range(4):
                    bi = i0b + u
                    rc = smallp.tile([128, 1], f32, tag="rc")
                    nc.vector.reciprocal(rc[:, :], po_ap(bi)[:, 128:129])
                    nc.vector.tensor_scalar_mul(
                        out_sb[:, bi, :], po_ap(bi)[:, 0:128], rc[:, :]
                    )
                nc.sync.dma_start(
                    od.ap()[h].rearrange("(n p) d -> p n d", p=128)[
                        :, i0b : i0b + 4, :
                    ],
                    out_sb[:, i0b : i0b + 4, :],
                )

    nc.finalize()
    return nc


def _get_nc():
    global _CACHED_NC
    if _CACHED_NC is None:
        _CACHED_NC = _build_nc()
    return _CACHED_NC


def run_sharded(k, v, trace=False):
    """k, v: [B*H, S, D] fp32. Returns (out [B*H, S, D], BassKernelResults)."""
    from concourse import bass_utils

    nc = _get_nc()
    in_maps = [
        {
            "k": np.ascontiguousarray(k[c * HPC : (c + 1) * HPC]),
            "v": np.ascontiguousarray(v[c * HPC : (c + 1) * HPC]),
        }
        for c in range(N_CORES)
    ]
    res = bass_utils.run_bass_kernel_spmd(
        nc, in_maps, core_ids=list(range(N_CORES)), trace=trace
    )
    out = np.concatenate([res.results[c]["out"] for c in range(N_CORES)], axis=0)
    return out, res


def kernel(q, k, v):
    k = np.asarray(k, dtype=np.float32).reshape(B * H, S, D)
    v = np.asarray(v, dtype=np.float32).reshape(B * H, S, D)
    out, _ = run_sharded(k, v, trace=False)
    return out.reshape(B, H, S, D)
